# revision 17
# baseline (speedup 1.0000x reference)
"""Trainium2 Bass kernel for agent attention (sparse_attention problem).

Per-core work (data-parallel over batch B=8 across 8 NeuronCores):
  x[b] [256, 64, 64] -> qkv 3x3 conv (dif-conv + BN folded into weights)
  -> agent attention (8 heads, d=32, 64 agent tokens)
  -> depthwise 3x3 pe conv on v -> 1x1 proj.

All heavy matmuls run in float32r (tf32-like, 1 cycle/row at free>=256).
"""
import numpy as np

NUM_HEADS = 8
AGENT_NUM = 64
THETA = 0.7
C = 256
H = W = 64
HW = H * W
D = C // NUM_HEADS          # 32
N_AG = AGENT_NUM            # 64
PS = 8                      # pool size
N_CORES = 8
B = 8

_cache = {}


def _build():
    import concourse.bass as bass
    import concourse.tile as tile
    from concourse import bacc, mybir

    f32 = mybir.dt.float32
    f32r = mybir.dt.float32r
    AF = mybir.ActivationFunctionType
    ALU = mybir.AluOpType
    AX = mybir.AxisListType

    nc = bacc.Bacc("TRN2", target_bir_lowering=False, debug=False,
                   enable_asserts=True, num_devices=N_CORES)

    # DRAM parameters (per core). float32r for anything feeding a matmul.
    X = nc.dram_tensor("x", [2, 128, H, W], f32r, kind="ExternalInput").ap()
    WQ = nc.dram_tensor("wq", [6, 128, 2, 9, 128], f32r,
                        kind="ExternalInput").ap()
    BQ = nc.dram_tensor("bq", [128, 6], f32, kind="ExternalInput").ap()
    PEW = nc.dram_tensor("pew", [128, 2, 9], f32, kind="ExternalInput").ap()
    PW = nc.dram_tensor("pw", [128, 2 * 256], f32r, kind="ExternalInput").ap()
    PB = nc.dram_tensor("pb", [128, 2], f32, kind="ExternalInput").ap()
    IDN = nc.dram_tensor("idn", [128, 128], f32r, kind="ExternalInput").ap()
    ZC = nc.dram_tensor("zc", [128, 4], f32r, kind="ExternalInput").ap()
    IND4 = nc.dram_tensor("ind4", [2, 256], f32, kind="ExternalInput").ap()
    OUT = nc.dram_tensor("out", [2, 128, HW], f32, kind="ExternalOutput").ap()

    # softmax exp scale: d^-0.5, with the 1/64 agent-pool mean folded in
    # (a tiles hold block *sums*).
    SCALE = (D ** -0.5) / (PS * PS)

    with tile.TileContext(nc) as tc:
        from contextlib import ExitStack
        with ExitStack() as top:
            # ---- persistent tiles (live across phases) ----
            pers = top.enter_context(tc.tile_pool(name="pers", bufs=1))
            q_sb = [pers.tile([128, HW], f32r, tag=f"q{i}", name=f"q{i}")
                    for i in range(2)]
            k_sb = [pers.tile([128, HW], f32r, tag=f"k{i}", name=f"k{i}")
                    for i in range(2)]
            v_pad = [pers.tile([128, 66 * 66], f32r, tag=f"vp{i}",
                               name=f"vp{i}") for i in range(2)]
            att_out = [pers.tile([128, HW], f32r, tag=f"ao{i}", name=f"ao{i}")
                       for i in range(2)]
            bq = pers.tile([128, 6], f32, tag="bq", name="bq")
            idn = pers.tile([128, 128], f32r, tag="idn", name="idn")
            pew = pers.tile([128, 2, 9], f32, tag="pew", name="pew")
            nc.sync.dma_start(bq[:], BQ[:])
            nc.sync.dma_start(idn[:], IDN[:])
            nc.sync.dma_start(pew[:], PEW[:])

            # zero v_pad borders (rows 0,65 and cols 0,65)
            for cc in range(2):
                vv = v_pad[cc][:].bitcast(f32).rearrange(
                    "p (r c) -> p r c", r=66, c=66)
                nc.vector.memset(vv[:, 0:1, :], 0.0)
                nc.vector.memset(vv[:, 65:66, :], 0.0)
                nc.vector.memset(vv[:, :, 0:1], 0.0)
                nc.vector.memset(vv[:, :, 65:66], 0.0)

            # =========== Phase A: qkv 3x3 conv (v first), pe-conv on DVE ====
            with ExitStack() as ph:
                cpool = ph.enter_context(tc.tile_pool(name="conv", bufs=1))
                wpool = ph.enter_context(tc.tile_pool(name="wq", bufs=2))
                cps = ph.enter_context(
                    tc.tile_pool(name="cps", bufs=8, space="PSUM"))

                x_pad = [cpool.tile([128, 66 * 66], f32r, tag=f"xp{i}",
                                    name=f"xp{i}") for i in range(2)]
                for kc in range(2):
                    xv = x_pad[kc][:].bitcast(f32).rearrange(
                        "p (r c) -> p r c", r=66, c=66)
                    nc.vector.memset(xv[:, 0:1, :], 0.0)
                    nc.vector.memset(xv[:, 65:66, :], 0.0)
                    nc.vector.memset(xv[:, :, 0:1], 0.0)
                    nc.vector.memset(xv[:, :, 65:66], 0.0)
                    xvr = x_pad[kc][:].rearrange(
                        "p (r c) -> p r c", r=66, c=66)
                    nc.sync.dma_start(xvr[:, 1:65, 1:65], X[kc])

                for mc in (4, 5, 0, 1, 2, 3):
                    wt = wpool.tile([128, 2, 9, 128], f32r, tag="w", name="w")
                    nc.sync.dma_start(wt[:], WQ[mc])
                    for rbg in range(2):
                        pst = [cps.tile([128, 512], f32, tag="cps",
                                        name="cpst") for _ in range(4)]
                        psvs = [t[:].rearrange("p (r c) -> p r c", r=8, c=64)
                                for t in pst]
                        i = 0
                        for kc in range(2):
                            xv = x_pad[kc][:].rearrange(
                                "p (r c) -> p r c", r=66, c=66)
                            for s in range(9):
                                ky, kx = s // 3, s % 3
                                for r4 in range(4):
                                    rb = 4 * rbg + r4
                                    rhs = xv[:, 8 * rb + ky: 8 * rb + ky + 8,
                                             kx: kx + 64]
                                    nc.tensor.matmul(
                                        psvs[r4], wt[:, kc, s, :], rhs,
                                        start=(i == 0), stop=(i == 17),
                                        skip_group_check=True)
                                i += 1
                        # copy out with bias, into q/k/v
                        bias = bq[:, mc: mc + 1]
                        for r4 in range(4):
                            rb = 4 * rbg + r4
                            if mc < 2:
                                dst = q_sb[mc][:, 512 * rb: 512 * (rb + 1)]
                                nc.vector.tensor_scalar_add(dst, pst[r4][:],
                                                            bias)
                            elif mc < 4:
                                dst = k_sb[mc - 2][:,
                                                   512 * rb: 512 * (rb + 1)]
                                nc.vector.tensor_scalar_add(dst, pst[r4][:],
                                                            bias)
                            else:
                                vv = v_pad[mc - 4][:].rearrange(
                                    "p (r c) -> p r c", r=66, c=66)
                                dst = vv[:, 8 * rb + 1: 8 * rb + 9, 1:65]
                                nc.vector.tensor_scalar_add(dst, psvs[r4],
                                                            bias)

                # depthwise 3x3 pe conv on DVE, accumulating into att_out.
                # Runs overlapped with the q/k conv matmuls (v was computed
                # first). Split into 4 row-groups per chunk for pipelining.
                for cc in range(2):
                    vvf = v_pad[cc][:].bitcast(f32).rearrange(
                        "p (r c) -> p r c", r=66, c=66)
                    aof = att_out[cc][:].rearrange(
                        "p (r c) -> p r c", r=64, c=64)
                    for g in range(4):
                        r0 = 16 * g
                        dst = aof[:, r0:r0 + 16, :]
                        for s in range(9):
                            ky, kx = s // 3, s % 3
                            sv = vvf[:, r0 + ky: r0 + ky + 16, kx: kx + 64]
                            if s == 0:
                                nc.vector.tensor_scalar_mul(
                                    dst, sv, pew[:, cc, 0:1])
                            else:
                                nc.vector.scalar_tensor_tensor(
                                    dst, sv, pew[:, cc, s:s + 1], dst,
                                    ALU.mult, ALU.add)

            # =========== Phase B: pooling + stage-1 attention ===========
            # pooled agent sums: a_sum[cc] [128, 64] (block sums of q)
            apool = top.enter_context(tc.tile_pool(name="apool", bufs=1))
            a_sum = [apool.tile([128, 64], f32, tag=f"as{i}", name=f"as{i}")
                     for i in range(2)]
            for ccq in range(2):
                qv = q_sb[ccq][:].rearrange(
                    "p (by dy bx dx) -> p by bx dy dx", by=8, dy=8, bx=8, dx=8)
                nc.vector.tensor_reduce(a_sum[ccq][:], qv, AX.XY, ALU.add)

            # 4-head block-diag a per chunk: [128, 256]; block j occupies
            # rows 32j..32j+32, cols 64j..64j+64 (same partitions as a_sum).
            a_bd4 = [apool.tile([128, 256], f32r, tag=f"abd{i}",
                                name=f"abd{i}") for i in range(2)]
            for cc in range(2):
                nc.vector.memset(a_bd4[cc][:].bitcast(f32), 0.0)
                for j in range(4):
                    nc.vector.tensor_copy(
                        a_bd4[cc][32 * j:32 * j + 32, 64 * j:64 * j + 64],
                        a_sum[cc][32 * j:32 * j + 32, :])

            # attnZ tiles [128, 64] (built after stage-1 agg)
            attnZ = [apool.tile([128, 64], f32r, tag=f"az{i}", name=f"az{i}")
                     for i in range(4)]

            with ExitStack() as ph:
                s1sb = ph.enter_context(tc.tile_pool(name="s1sb", bufs=3))
                tr_ps = ph.enter_context(
                    tc.tile_pool(name="trps", bufs=2, space="PSUM"))
                st_ps = ph.enter_context(
                    tc.tile_pool(name="stps", bufs=2, space="PSUM"))
                at_ps = ph.enter_context(
                    tc.tile_pool(name="atps", bufs=4, space="PSUM"))

                attn_ps = [at_ps.tile([128, 66], f32, tag="at", name="at")
                           for _ in range(4)]

                for ch in range(32):
                    # vT for this 128-pixel chunk: [128, 4*66] pair-major,
                    # cols 64,65 of each block = ones (col 64 -> Z1)
                    vt = s1sb.tile([128, 4 * 66], f32r, tag="vt", name="vt")
                    vtv = vt[:].bitcast(f32).rearrange(
                        "p (a b) -> p a b", a=4, b=66)
                    nc.vector.memset(vtv[:, :, 64:66], 1.0)
                    for cc in range(2):
                        vv = v_pad[cc][:].rearrange(
                            "p (r c) -> p r c", r=66, c=66)
                        vstg = s1sb.tile([128, 128], f32r, tag="vstg",
                                         name="vstg")
                        nc.vector.tensor_copy(
                            vstg[:].rearrange("p (r c) -> p r c", r=2, c=64),
                            vv[:, 2 * ch + 1: 2 * ch + 3, 1:65])
                        tp = tr_ps.tile([128, 128], f32, tag="tr", name="trt")
                        nc.tensor.transpose(tp[:].bitcast(f32r), vstg[:],
                                            idn[:])
                        nc.vector.tensor_copy(
                            vt[:, (2 * cc) * 66:(2 * cc) * 66 + 64],
                            tp[:, 0:64])
                        nc.vector.tensor_copy(
                            vt[:, (2 * cc + 1) * 66:(2 * cc + 1) * 66 + 64],
                            tp[:, 64:128])
                    for cc in range(2):
                        sp = st_ps.tile([128, 256], f32, tag="st", name="stt")
                        nc.tensor.matmul(sp[:],
                                         k_sb[cc][:, 128 * ch:128 * (ch + 1)],
                                         a_bd4[cc][:], start=True, stop=True)
                        et = s1sb.tile([128, 256], f32r, tag="et", name="et")
                        nc.scalar.activation(et[:], sp[:], AF.Exp, scale=SCALE)
                        for half in range(2):
                            hp = 2 * cc + half
                            nc.tensor.matmul(
                                attn_ps[hp][:],
                                et[:, 128 * half:128 * (half + 1)],
                                vt[:, 66 * hp:66 * (hp + 1)],
                                start=(ch == 0), stop=(ch == 31))

                # normalize stage-1 output rows by Z1, build attnZ
                for hp in range(4):
                    r1 = s1sb.tile([128, 1], f32, tag="r1", name="r1")
                    nc.vector.reciprocal(r1[:], attn_ps[hp][:, 64:65])
                    nc.vector.memset(attnZ[hp][:].bitcast(f32), 0.0)
                    nc.vector.tensor_scalar_mul(
                        attnZ[hp][0:64, 0:32], attn_ps[hp][0:64, 0:32],
                        r1[0:64, :])
                    nc.vector.tensor_scalar_mul(
                        attnZ[hp][64:128, 32:64], attn_ps[hp][64:128, 32:64],
                        r1[64:128, :])

            # =========== Phase C: stage-2 attention + proj (fused) =======
            zc = apool.tile([128, 4], f32r, tag="zc", name="zc")
            ind4 = apool.tile([2, 256], f32, tag="ind4", name="ind4")
            pw = apool.tile([128, 2 * 256], f32r, tag="pw", name="pwt")
            pb = apool.tile([128, 2], f32, tag="pb", name="pbt")
            nc.sync.dma_start(zc[:], ZC[:])
            nc.sync.dma_start(ind4[:], IND4[:])
            nc.sync.dma_start(pw[:], PW[:])
            nc.sync.dma_start(pb[:], PB[:])
            pwv = pw[:].rearrange("p (a b) -> p a b", a=2, b=256)

            with ExitStack() as ph:
                s2sb = ph.enter_context(tc.tile_pool(name="s2sb", bufs=3))
                osb = ph.enter_context(tc.tile_pool(name="osb", bufs=3))
                s2_ps = ph.enter_context(
                    tc.tile_pool(name="s2ps", bufs=2, space="PSUM"))
                u_ps = ph.enter_context(
                    tc.tile_pool(name="ups", bufs=2, space="PSUM"))
                z_ps = ph.enter_context(
                    tc.tile_pool(name="zps", bufs=1, space="PSUM"))
                b_ps = ph.enter_context(
                    tc.tile_pool(name="bps", bufs=1, space="PSUM"))
                pr_ps = ph.enter_context(
                    tc.tile_pool(name="prps", bufs=1, space="PSUM"))

                for nt in range(8):
                    for cc in range(2):
                        up = u_ps.tile([128, 512], f32, tag="u", name="u")
                        zp1 = z_ps.tile([2, 512], f32, tag="z1", name="z1")
                        zp2 = z_ps.tile([2, 512], f32, tag="z2", name="z2")
                        e2s = []
                        for half in range(2):
                            hp = 2 * cc + half
                            sp = s2_ps.tile([128, 512], f32, tag="s2",
                                            name="s2t")
                            nc.tensor.matmul(
                                sp[:],
                                a_bd4[cc][:, 128 * half:128 * (half + 1)],
                                q_sb[cc][:, 512 * nt:512 * (nt + 1)],
                                start=True, stop=True)
                            e2 = s2sb.tile([128, 512], f32r, tag="e2",
                                           name="e2")
                            nc.scalar.activation(e2[:], sp[:], AF.Exp,
                                                 scale=SCALE)
                            e2s.append(e2)
                            if half == 0:
                                nc.tensor.matmul(up[0:64, :], attnZ[hp][:],
                                                 e2[:], start=True, stop=True)
                            else:
                                nc.tensor.matmul(
                                    up[64:128, :],
                                    attnZ[hp][:].bitcast(f32),
                                    e2[:].bitcast(f32),
                                    start=True, stop=True)
                        nc.tensor.matmul(zp1[:], zc[:, 0:2], e2s[0][:],
                                         start=True, stop=True)
                        nc.tensor.matmul(zp2[:], zc[:, 2:4], e2s[1][:],
                                         start=True, stop=True)
                        rz1 = s2sb.tile([2, 512], f32, tag="rz1", name="rz1")
                        rz2 = s2sb.tile([2, 512], f32, tag="rz2", name="rz2")
                        nc.vector.reciprocal_approx_fast(rz1[:], zp1[:])
                        nc.vector.reciprocal_approx_fast(rz2[:], zp2[:])
                        bp = b_ps.tile([128, 512], f32, tag="b", name="bt")
                        nc.tensor.matmul(bp[:], ind4[:, 0:128],
                                         rz1[:], start=True, stop=False)
                        nc.tensor.matmul(bp[:], ind4[:, 128:256],
                                         rz2[:], start=False, stop=True)
                        bc = s2sb.tile([128, 512], f32, tag="bc", name="bct")
                        nc.vector.tensor_copy(bc[:], bp[:])
                        tmp = s2sb.tile([128, 512], f32, tag="tmp",
                                        name="tmp")
                        nc.vector.tensor_tensor(tmp[:], up[:], bc[:],
                                                ALU.mult)
                        sl = att_out[cc][:, 512 * nt:512 * (nt + 1)]
                        nc.vector.tensor_tensor(sl, tmp[:], sl.bitcast(f32),
                                                ALU.add)
                    # fused 1x1 proj for this spatial tile
                    for mc in range(2):
                        pp = pr_ps.tile([128, 512], f32, tag="pr", name="prt")
                        for kc in range(2):
                            nc.tensor.matmul(
                                pp[:], pwv[:, kc, 128 * mc:128 * (mc + 1)],
                                att_out[kc][:, 512 * nt:512 * (nt + 1)],
                                start=(kc == 0), stop=(kc == 1))
                        ot = osb.tile([128, 512], f32, tag="ot", name="ott")
                        nc.vector.tensor_scalar_add(ot[:], pp[:],
                                                    pb[:, mc:mc + 1])
                        nc.sync.dma_start(
                            OUT[mc, :, 512 * nt:512 * (nt + 1)], ot[:])

    nc.compile()
    return nc


def _prep_consts(qkv_w, qkv_s, qkv_b, pe_w, pe_s, pe_b, proj_w, proj_s,
                 proj_b):
    f = np.float32
    w = np.asarray(qkv_w, f).copy()          # [768, 256, 3, 3]
    dif = (w[:, :, 0, 1] + w[:, :, 1, 0] + w[:, :, 1, 1] + w[:, :, 1, 2]
           + w[:, :, 2, 1])
    w[:, :, 1, 1] -= THETA * dif
    w *= np.asarray(qkv_s, f)[:, None, None, None]
    # WQ[mc, p, kc, s, o'] = w[128*mc+o', 128*kc+p, s//3, s%3]
    wq = w.reshape(6, 128, 2, 128, 9)        # [mc, o', kc, p, s]
    wq = np.ascontiguousarray(wq.transpose(0, 3, 2, 4, 1))  # [6,128,2,9,128]

    bq = np.ascontiguousarray(np.asarray(qkv_b, f).reshape(6, 128).T)

    pe_wf = np.asarray(pe_w, f)[:, 0] * np.asarray(pe_s, f)[:, None, None]
    pew = np.zeros((128, 2, 9), f)
    for kc in range(2):
        for s in range(9):
            pew[:, kc, s] = pe_wf[128 * kc:128 * (kc + 1), s // 3, s % 3]

    pwm = np.asarray(proj_w, f)[:, :, 0, 0] * np.asarray(proj_s, f)[:, None]
    pw = np.ascontiguousarray(
        pwm.T.reshape(2, 128, 256).transpose(1, 0, 2).reshape(128, 512))
    pbv = np.asarray(proj_b, f) + pwm @ np.asarray(pe_b, f)
    pb = np.ascontiguousarray(pbv.reshape(2, 128).T)

    idn = np.eye(128, dtype=f)
    zc = np.zeros((128, 4), f)
    zc[0:64, 0] = 1.0
    zc[64:128, 1] = 1.0
    zc[0:64, 2] = 1.0
    zc[64:128, 3] = 1.0
    ind4 = np.zeros((2, 256), f)
    ind4[0, 0:32] = 1.0
    ind4[1, 32:64] = 1.0
    ind4[0, 128 + 64:128 + 96] = 1.0
    ind4[1, 128 + 96:128 + 128] = 1.0
    return dict(wq=wq, bq=bq, pew=pew, pw=pw, pb=pb, idn=idn, zc=zc,
                ind4=ind4)


def kernel(x, qkv_w, qkv_s, qkv_b, pe_w, pe_s, pe_b, proj_w, proj_s, proj_b):
    from concourse.bass_utils import run_bass_kernel_spmd

    if "nc" not in _cache:
        _cache["nc"] = _build()
    nc = _cache["nc"]

    consts = _prep_consts(qkv_w, qkv_s, qkv_b, pe_w, pe_s, pe_b, proj_w,
                          proj_s, proj_b)
    x = np.asarray(x, np.float32)
    in_maps = []
    for b in range(B):
        m = dict(consts)
        m["x"] = np.ascontiguousarray(x[b].reshape(2, 128, H, W))
        in_maps.append(m)

    res = run_bass_kernel_spmd(nc, in_maps, list(range(N_CORES)), trace=False)
    out = np.empty((B, C, H, W), np.float32)
    for b in range(B):
        out[b] = res.results[b]["out"].reshape(C, H, W)
    return out


# revision 19
# speedup vs baseline: 1.0506x; 1.0506x over previous
"""Trainium2 Bass kernel for agent attention (sparse_attention problem).

Per-core work (data-parallel over batch B=8 across 8 NeuronCores):
  x[b] [256, 64, 64] -> qkv 3x3 conv (dif-conv + BN folded into weights)
  -> agent attention (8 heads, d=32, 64 agent tokens)
  -> depthwise 3x3 pe conv on v -> 1x1 proj.

All heavy matmuls run in float32r (tf32-like, 1 cycle/row at free>=256).
"""
import numpy as np

NUM_HEADS = 8
AGENT_NUM = 64
THETA = 0.7
C = 256
H = W = 64
HW = H * W
D = C // NUM_HEADS          # 32
N_AG = AGENT_NUM            # 64
PS = 8                      # pool size
N_CORES = 8
B = 8

_cache = {}


def _build():
    import concourse.bass as bass
    import concourse.tile as tile
    from concourse import bacc, mybir

    f32 = mybir.dt.float32
    f32r = mybir.dt.float32r
    AF = mybir.ActivationFunctionType
    ALU = mybir.AluOpType
    AX = mybir.AxisListType

    nc = bacc.Bacc("TRN2", target_bir_lowering=False, debug=False,
                   enable_asserts=True, num_devices=N_CORES)

    # DRAM parameters (per core). float32r for anything feeding a matmul.
    X = nc.dram_tensor("x", [2, 128, H, W], f32r, kind="ExternalInput").ap()
    WQ = nc.dram_tensor("wq", [6, 128, 2, 9, 128], f32r,
                        kind="ExternalInput").ap()
    BQ = nc.dram_tensor("bq", [128, 6], f32, kind="ExternalInput").ap()
    PEW = nc.dram_tensor("pew", [128, 2, 9], f32, kind="ExternalInput").ap()
    PW = nc.dram_tensor("pw", [128, 2 * 256], f32r, kind="ExternalInput").ap()
    PB = nc.dram_tensor("pb", [128, 2], f32, kind="ExternalInput").ap()
    IDN = nc.dram_tensor("idn", [128, 128], f32r, kind="ExternalInput").ap()
    OUT = nc.dram_tensor("out", [2, 128, HW], f32, kind="ExternalOutput").ap()

    # softmax exp scale: d^-0.5, with the 1/64 agent-pool mean folded in
    # (a tiles hold block *sums*).
    SCALE = (D ** -0.5) / (PS * PS)

    with tile.TileContext(nc) as tc:
        from contextlib import ExitStack
        with ExitStack() as top:
            # ---- persistent tiles (live across phases) ----
            pers = top.enter_context(tc.tile_pool(name="pers", bufs=1))
            q_sb = [pers.tile([128, HW], f32r, tag=f"q{i}", name=f"q{i}")
                    for i in range(2)]
            k_sb = [pers.tile([128, HW], f32r, tag=f"k{i}", name=f"k{i}")
                    for i in range(2)]
            v_pad = [pers.tile([128, 66 * 66], f32r, tag=f"vp{i}",
                               name=f"vp{i}") for i in range(2)]
            att_out = [pers.tile([128, HW], f32r, tag=f"ao{i}", name=f"ao{i}")
                       for i in range(2)]
            bq = pers.tile([128, 6], f32, tag="bq", name="bq")
            idn = pers.tile([128, 128], f32r, tag="idn", name="idn")
            pew = pers.tile([128, 2, 9], f32, tag="pew", name="pew")
            nc.sync.dma_start(bq[:], BQ[:])
            nc.sync.dma_start(idn[:], IDN[:])
            nc.sync.dma_start(pew[:], PEW[:])

            # zero v_pad borders (rows 0,65 and cols 0,65)
            for cc in range(2):
                vv = v_pad[cc][:].bitcast(f32).rearrange(
                    "p (r c) -> p r c", r=66, c=66)
                nc.vector.memset(vv[:, 0:1, :], 0.0)
                nc.vector.memset(vv[:, 65:66, :], 0.0)
                nc.vector.memset(vv[:, :, 0:1], 0.0)
                nc.vector.memset(vv[:, :, 65:66], 0.0)

            # =========== Phase A: qkv 3x3 conv (v first), pe-conv on DVE ====
            with ExitStack() as ph:
                cpool = ph.enter_context(tc.tile_pool(name="conv", bufs=1))
                wpool = ph.enter_context(tc.tile_pool(name="wq", bufs=2))
                cps = ph.enter_context(
                    tc.tile_pool(name="cps", bufs=8, space="PSUM"))

                x_pad = [cpool.tile([128, 66 * 66], f32r, tag=f"xp{i}",
                                    name=f"xp{i}") for i in range(2)]
                for kc in range(2):
                    xv = x_pad[kc][:].bitcast(f32).rearrange(
                        "p (r c) -> p r c", r=66, c=66)
                    nc.vector.memset(xv[:, 0:1, :], 0.0)
                    nc.vector.memset(xv[:, 65:66, :], 0.0)
                    nc.vector.memset(xv[:, :, 0:1], 0.0)
                    nc.vector.memset(xv[:, :, 65:66], 0.0)
                    xvr = x_pad[kc][:].rearrange(
                        "p (r c) -> p r c", r=66, c=66)
                    nc.sync.dma_start(xvr[:, 1:65, 1:65], X[kc])

                for mc in (4, 5, 0, 1, 2, 3):
                    wt = wpool.tile([128, 2, 9, 128], f32r, tag="w", name="w")
                    nc.sync.dma_start(wt[:], WQ[mc])
                    for rbg in range(2):
                        pst = [cps.tile([128, 512], f32, tag="cps",
                                        name="cpst") for _ in range(4)]
                        psvs = [t[:].rearrange("p (r c) -> p r c", r=8, c=64)
                                for t in pst]
                        i = 0
                        for kc in range(2):
                            xv = x_pad[kc][:].rearrange(
                                "p (r c) -> p r c", r=66, c=66)
                            for s in range(9):
                                ky, kx = s // 3, s % 3
                                for r4 in range(4):
                                    rb = 4 * rbg + r4
                                    rhs = xv[:, 8 * rb + ky: 8 * rb + ky + 8,
                                             kx: kx + 64]
                                    nc.tensor.matmul(
                                        psvs[r4], wt[:, kc, s, :], rhs,
                                        start=(i == 0), stop=(i == 17),
                                        skip_group_check=True)
                                i += 1
                        # copy out with bias, into q/k/v
                        bias = bq[:, mc: mc + 1]
                        for r4 in range(4):
                            rb = 4 * rbg + r4
                            if mc < 2:
                                dst = q_sb[mc][:, 512 * rb: 512 * (rb + 1)]
                                nc.vector.tensor_scalar_add(dst, pst[r4][:],
                                                            bias)
                            elif mc < 4:
                                dst = k_sb[mc - 2][:,
                                                   512 * rb: 512 * (rb + 1)]
                                nc.vector.tensor_scalar_add(dst, pst[r4][:],
                                                            bias)
                            else:
                                vv = v_pad[mc - 4][:].rearrange(
                                    "p (r c) -> p r c", r=66, c=66)
                                dst = vv[:, 8 * rb + 1: 8 * rb + 9, 1:65]
                                nc.vector.tensor_scalar_add(dst, psvs[r4],
                                                            bias)

                # depthwise 3x3 pe conv on DVE, accumulating into att_out.
                # Runs overlapped with the q/k conv matmuls (v was computed
                # first). Split into 4 row-groups per chunk for pipelining.
                for cc in range(2):
                    vvf = v_pad[cc][:].bitcast(f32).rearrange(
                        "p (r c) -> p r c", r=66, c=66)
                    aof = att_out[cc][:].rearrange(
                        "p (r c) -> p r c", r=64, c=64)
                    for g in range(4):
                        r0 = 16 * g
                        dst = aof[:, r0:r0 + 16, :]
                        for s in range(9):
                            ky, kx = s // 3, s % 3
                            sv = vvf[:, r0 + ky: r0 + ky + 16, kx: kx + 64]
                            if s == 0:
                                nc.vector.tensor_scalar_mul(
                                    dst, sv, pew[:, cc, 0:1])
                            else:
                                nc.vector.scalar_tensor_tensor(
                                    dst, sv, pew[:, cc, s:s + 1], dst,
                                    ALU.mult, ALU.add)

            # =========== Phase B: pooling + stage-1 attention ===========
            # pooled agent sums: a_sum[cc] [128, 64] (block sums of q)
            apool = top.enter_context(tc.tile_pool(name="apool", bufs=1))
            a_sum = [apool.tile([128, 64], f32, tag=f"as{i}", name=f"as{i}")
                     for i in range(2)]
            for ccq in range(2):
                qv = q_sb[ccq][:].rearrange(
                    "p (by dy bx dx) -> p by bx dy dx", by=8, dy=8, bx=8, dx=8)
                nc.vector.tensor_reduce(a_sum[ccq][:], qv, AX.XY, ALU.add)

            # 4-head block-diag a per chunk: [128, 256]; block j occupies
            # rows 32j..32j+32, cols 64j..64j+64 (same partitions as a_sum).
            a_bd4 = [apool.tile([128, 256], f32r, tag=f"abd{i}",
                                name=f"abd{i}") for i in range(2)]
            for cc in range(2):
                nc.vector.memset(a_bd4[cc][:].bitcast(f32), 0.0)
                for j in range(4):
                    nc.vector.tensor_copy(
                        a_bd4[cc][32 * j:32 * j + 32, 64 * j:64 * j + 64],
                        a_sum[cc][32 * j:32 * j + 32, :])

            # attnZ tiles [128, 64] (built after stage-1 agg)
            attnZ = [apool.tile([128, 68], f32r, tag=f"az{i}", name=f"az{i}")
                     for i in range(4)]

            with ExitStack() as ph:
                s1sb = ph.enter_context(tc.tile_pool(name="s1sb", bufs=3))
                tr_ps = ph.enter_context(
                    tc.tile_pool(name="trps", bufs=2, space="PSUM"))
                st_ps = ph.enter_context(
                    tc.tile_pool(name="stps", bufs=2, space="PSUM"))
                at_ps = ph.enter_context(
                    tc.tile_pool(name="atps", bufs=4, space="PSUM"))

                attn_ps = [at_ps.tile([128, 66], f32, tag="at", name="at")
                           for _ in range(4)]

                for ch in range(32):
                    # vT for this 128-pixel chunk: [128, 4*66] pair-major,
                    # cols 64,65 of each block = ones (col 64 -> Z1)
                    vt = s1sb.tile([128, 4 * 66], f32r, tag="vt", name="vt")
                    vtv = vt[:].bitcast(f32).rearrange(
                        "p (a b) -> p a b", a=4, b=66)
                    nc.vector.memset(vtv[:, :, 64:66], 1.0)
                    for cc in range(2):
                        vv = v_pad[cc][:].rearrange(
                            "p (r c) -> p r c", r=66, c=66)
                        vstg = s1sb.tile([128, 128], f32r, tag="vstg",
                                         name="vstg")
                        nc.vector.tensor_copy(
                            vstg[:].rearrange("p (r c) -> p r c", r=2, c=64),
                            vv[:, 2 * ch + 1: 2 * ch + 3, 1:65])
                        tp = tr_ps.tile([128, 128], f32, tag="tr", name="trt")
                        nc.tensor.transpose(tp[:].bitcast(f32r), vstg[:],
                                            idn[:])
                        nc.vector.tensor_copy(
                            vt[:, (2 * cc) * 66:(2 * cc) * 66 + 64],
                            tp[:, 0:64])
                        nc.vector.tensor_copy(
                            vt[:, (2 * cc + 1) * 66:(2 * cc + 1) * 66 + 64],
                            tp[:, 64:128])
                    for cc in range(2):
                        sp = st_ps.tile([128, 256], f32, tag="st", name="stt")
                        nc.tensor.matmul(sp[:],
                                         k_sb[cc][:, 128 * ch:128 * (ch + 1)],
                                         a_bd4[cc][:], start=True, stop=True)
                        et = s1sb.tile([128, 256], f32r, tag="et", name="et")
                        nc.scalar.activation(et[:], sp[:], AF.Exp, scale=SCALE)
                        for half in range(2):
                            hp = 2 * cc + half
                            nc.tensor.matmul(
                                attn_ps[hp][:],
                                et[:, 128 * half:128 * (half + 1)],
                                vt[:, 66 * hp:66 * (hp + 1)],
                                start=(ch == 0), stop=(ch == 31))

                # normalize stage-1 output rows by Z1, build attnZ
                for hp in range(4):
                    r1 = s1sb.tile([128, 1], f32, tag="r1", name="r1")
                    nc.vector.reciprocal(r1[:], attn_ps[hp][:, 64:65])
                    nc.vector.memset(attnZ[hp][:].bitcast(f32), 0.0)
                    nc.vector.memset(attnZ[hp][0:64, 64:65].bitcast(f32), 1.0)
                    nc.vector.memset(attnZ[hp][64:128, 65:66].bitcast(f32),
                                     1.0)
                    nc.vector.tensor_scalar_mul(
                        attnZ[hp][0:64, 0:32], attn_ps[hp][0:64, 0:32],
                        r1[0:64, :])
                    nc.vector.tensor_scalar_mul(
                        attnZ[hp][64:128, 32:64], attn_ps[hp][64:128, 32:64],
                        r1[64:128, :])

            # =========== Phase C: stage-2 attention + proj (fused) =======
            # Transposed aggregation: outT[N, d(+Z cols)] = e2.T @ attnZ.
            # Z lands per-partition -> cheap normalize, then PE-transpose
            # back to [d, N], folding the pe-conv add into the PSUM drain.
            pw = apool.tile([128, 2 * 256], f32r, tag="pw", name="pwt")
            pb = apool.tile([128, 2], f32, tag="pb", name="pbt")
            nc.sync.dma_start(pw[:], PW[:])
            nc.sync.dma_start(pb[:], PB[:])
            pwv = pw[:].rearrange("p (a b) -> p a b", a=2, b=256)

            with ExitStack() as ph:
                s2sb = ph.enter_context(tc.tile_pool(name="s2sb", bufs=4))
                osb = ph.enter_context(tc.tile_pool(name="osb", bufs=3))
                s2_ps = ph.enter_context(
                    tc.tile_pool(name="s2ps", bufs=2, space="PSUM"))
                g_ps = ph.enter_context(
                    tc.tile_pool(name="gps", bufs=3, space="PSUM"))
                t_ps = ph.enter_context(
                    tc.tile_pool(name="tps", bufs=2, space="PSUM"))
                pr_ps = ph.enter_context(
                    tc.tile_pool(name="prps", bufs=1, space="PSUM"))

                for nt in range(8):
                    for cc in range(2):
                        e2s = []
                        for half in range(2):
                            sp = s2_ps.tile([128, 512], f32, tag="s2",
                                            name="s2t")
                            nc.tensor.matmul(
                                sp[:],
                                a_bd4[cc][:, 128 * half:128 * (half + 1)],
                                q_sb[cc][:, 512 * nt:512 * (nt + 1)],
                                start=True, stop=True)
                            e2 = s2sb.tile([128, 512], f32r, tag="e2",
                                           name="e2")
                            nc.scalar.activation(e2[:], sp[:], AF.Exp,
                                                 scale=SCALE)
                            e2s.append(e2)
                        for sub in range(4):
                            res = s2sb.tile([128, 128], f32r, tag="res",
                                            name="res")
                            for half in range(2):
                                hp = 2 * cc + half
                                gp = g_ps.tile([128, 68], f32, tag="g",
                                               name="gt")
                                nc.tensor.matmul(
                                    gp[:],
                                    e2s[half][:, 128 * sub:128 * (sub + 1)],
                                    attnZ[hp][:], start=True, stop=True)
                                r2 = s2sb.tile([128, 2], f32, tag="r2",
                                               name="r2")
                                nc.vector.reciprocal(r2[:], gp[:, 64:66])
                                nc.vector.tensor_scalar_mul(
                                    res[:, 64 * half:64 * half + 32],
                                    gp[:, 0:32], r2[:, 0:1])
                                nc.vector.tensor_scalar_mul(
                                    res[:, 64 * half + 32:64 * half + 64],
                                    gp[:, 32:64], r2[:, 1:2])
                            tp = t_ps.tile([128, 128], f32, tag="tp",
                                           name="tpt")
                            nc.tensor.transpose(tp[:].bitcast(f32r), res[:],
                                                idn[:])
                            # drain transposed tile, adding the pe-conv term
                            sl = att_out[cc][:, 512 * nt + 128 * sub:
                                             512 * nt + 128 * (sub + 1)]
                            nc.vector.tensor_tensor(sl, tp[:],
                                                    sl.bitcast(f32), ALU.add)
                    # fused 1x1 proj for this spatial tile
                    for mc in range(2):
                        pp = pr_ps.tile([128, 512], f32, tag="pr", name="prt")
                        for kc in range(2):
                            nc.tensor.matmul(
                                pp[:], pwv[:, kc, 128 * mc:128 * (mc + 1)],
                                att_out[kc][:, 512 * nt:512 * (nt + 1)],
                                start=(kc == 0), stop=(kc == 1))
                        ot = osb.tile([128, 512], f32, tag="ot", name="ott")
                        nc.vector.tensor_scalar_add(ot[:], pp[:],
                                                    pb[:, mc:mc + 1])
                        nc.sync.dma_start(
                            OUT[mc, :, 512 * nt:512 * (nt + 1)], ot[:])

    nc.compile()
    return nc


def _prep_consts(qkv_w, qkv_s, qkv_b, pe_w, pe_s, pe_b, proj_w, proj_s,
                 proj_b):
    f = np.float32
    w = np.asarray(qkv_w, f).copy()          # [768, 256, 3, 3]
    dif = (w[:, :, 0, 1] + w[:, :, 1, 0] + w[:, :, 1, 1] + w[:, :, 1, 2]
           + w[:, :, 2, 1])
    w[:, :, 1, 1] -= THETA * dif
    w *= np.asarray(qkv_s, f)[:, None, None, None]
    # WQ[mc, p, kc, s, o'] = w[128*mc+o', 128*kc+p, s//3, s%3]
    wq = w.reshape(6, 128, 2, 128, 9)        # [mc, o', kc, p, s]
    wq = np.ascontiguousarray(wq.transpose(0, 3, 2, 4, 1))  # [6,128,2,9,128]

    bq = np.ascontiguousarray(np.asarray(qkv_b, f).reshape(6, 128).T)

    pe_wf = np.asarray(pe_w, f)[:, 0] * np.asarray(pe_s, f)[:, None, None]
    pew = np.zeros((128, 2, 9), f)
    for kc in range(2):
        for s in range(9):
            pew[:, kc, s] = pe_wf[128 * kc:128 * (kc + 1), s // 3, s % 3]

    pwm = np.asarray(proj_w, f)[:, :, 0, 0] * np.asarray(proj_s, f)[:, None]
    pw = np.ascontiguousarray(
        pwm.T.reshape(2, 128, 256).transpose(1, 0, 2).reshape(128, 512))
    pbv = np.asarray(proj_b, f) + pwm @ np.asarray(pe_b, f)
    pb = np.ascontiguousarray(pbv.reshape(2, 128).T)

    idn = np.eye(128, dtype=f)
    return dict(wq=wq, bq=bq, pew=pew, pw=pw, pb=pb, idn=idn)


def kernel(x, qkv_w, qkv_s, qkv_b, pe_w, pe_s, pe_b, proj_w, proj_s, proj_b):
    from concourse.bass_utils import run_bass_kernel_spmd

    if "nc" not in _cache:
        _cache["nc"] = _build()
    nc = _cache["nc"]

    consts = _prep_consts(qkv_w, qkv_s, qkv_b, pe_w, pe_s, pe_b, proj_w,
                          proj_s, proj_b)
    x = np.asarray(x, np.float32)
    in_maps = []
    for b in range(B):
        m = dict(consts)
        m["x"] = np.ascontiguousarray(x[b].reshape(2, 128, H, W))
        in_maps.append(m)

    res = run_bass_kernel_spmd(nc, in_maps, list(range(N_CORES)), trace=False)
    out = np.empty((B, C, H, W), np.float32)
    for b in range(B):
        out[b] = res.results[b]["out"].reshape(C, H, W)
    return out
